# revision 9
# baseline (speedup 1.0000x reference)
"""CTC loss on 8 trn2 NeuronCores.

Design:
- Batch B=64 split 8/core for the memory-bound part: each core streams its
  own 27MB of predicts through ACT exp(+accum) for the log_softmax
  denominators, which factor out of the CTC DP entirely
  (loss = -(ln L + renorms - sum_t ln denom_t)). Raw per-(t,b) denominator
  sums ship back to host; ln+sum over t happens there (keeps the ACT
  engine on the exp table set the whole kernel - no table-load thrash).
- The T=128-step CTC DP runs in linear space with sparse renorms (k=21,42
  + pre-final; f32 headroom easily covers ~5x/step growth over 21 steps).
  The serial chain is split in half across core pairs: even cores run the
  FORWARD chain for the pair's 16 samples, odd cores the BACKWARD
  (suffix) chain, both as the *identical* SPMD program - the direction
  lives entirely in host-prepared data (s-axis reversed for backward,
  transition masks baked in as zeroed linear factors, E_127 absorbed into
  the backward init). Chain factors are exp'd on host and DMA'd in ready
  to use, so the chain starts ~3us in and hides fully under the stream.
- Both chains are 63 steps of 3 fused DVE ops; cores return chain states
  via DMA and the host combines L = sum_s alpha_63[s] * gamma_63[s].
"""

from contextlib import ExitStack

import numpy as np

import concourse.bacc as bacc
import concourse.tile as tile
import concourse.mybir as mybir
from concourse.ap import AP
from concourse.bass_utils import run_bass_kernel_spmd

B, T, C, L = 64, 128, 6625, 25
S = 2 * L + 1  # 51
M = 8          # cores
BS = B // M    # own samples per core (denominator stream)
PS = 2 * BS    # pair samples per core (DP chain)
NSTEP = 63
NSLOT = 64     # 63 steps + final-multiply slot
RENORM = 21    # in-chain renorm every 21 steps (k=21, 42)
NREN = 3       # 2 in-chain renorms + 1 pre-final
# Last sample streams in shrinking column chunks so the final exp (which
# can only start once the last bytes land) is short.
TAILW = [2625, 1700, 1500, 800]
NCOL = BS - 1 + len(TAILW)  # denominator accumulator columns
F32 = mybir.dt.float32

_cached = {}


def _dup_free(ap, n):
    """AP reading the free range of `ap` n times: [.., (0,n), (step,cnt)]."""
    dims = [list(d) for d in ap.ap]
    return AP(ap.tensor, ap.offset, dims[:-1] + [[0, n]] + [dims[-1]])


def _strided2(ap, gap, n):
    """AP over `ap`'s tensor writing two n-wide blocks `gap` apart."""
    dims = [list(d) for d in ap.ap]
    return AP(ap.tensor, ap.offset, dims[:-1] + [[gap, 2], [1, n]])


def _build():
    if "nc" in _cached:
        return _cached["nc"]
    nc = bacc.Bacc(
        "TRN2", target_bir_lowering=False, debug=False, num_devices=M
    )
    x = nc.dram_tensor("x", [BS, T, C], F32, kind="ExternalInput").ap()
    ecat = nc.dram_tensor("ecat", [PS, NSLOT * 2 * S], F32,
                          kind="ExternalInput").ap()
    yinit = nc.dram_tensor("yinit", [PS, S], F32, kind="ExternalInput").ap()
    xpk = nc.dram_tensor("xpk", [PS, S + NREN], F32, kind="ExternalOutput").ap()
    dsum = nc.dram_tensor("dsum", [T, NCOL], F32, kind="ExternalOutput").ap()

    EXP = mybir.ActivationFunctionType.Exp
    MULT = mybir.AluOpType.mult

    with tile.TileContext(nc) as tc, ExitStack() as ctx:
        cpool = ctx.enter_context(tc.tile_pool(name="consts", bufs=1))
        xpool = ctx.enter_context(tc.tile_pool(name="xs", bufs=4))

        # --- chain factors + init first (chain hides under the stream) ---
        et = cpool.tile([PS, NSLOT * 2 * S], F32)
        nc.sync.dma_start(et[:], ecat)
        y_sb = cpool.tile([PS, S], F32)
        nc.sync.dma_start(y_sb[:], yinit)

        # --- arm the stream: 7 full samples + shrinking tail chunks ---
        xts = []
        for b in range(BS - 1):
            xt = xpool.tile([T, C], F32, tag="xt")
            nc.sync.dma_start(xt[:], x[b])
            xts.append(xt)
        c0 = 0
        for cw in TAILW:
            xt = xpool.tile([T, cw], F32, tag="xt")
            nc.sync.dma_start(xt[:], x[BS - 1, :, c0 : c0 + cw])
            xts.append(xt)
            c0 += cw

        # --- DP chain: 63 steps of 3 fused DVE ops ---
        # wcat layout: [pad2 | w(51) | pad2 | wc(51)] = 106 cols
        wcat = cpool.tile([PS, 2 * S + 4], F32)
        u_t = cpool.tile([PS, S], F32)
        xpack = cpool.tile([PS, S + NREN], F32)  # [X(51) | ys(NREN)]
        inv = cpool.tile([PS, 1], F32)
        nc.vector.memset(wcat[:], 0.0)

        w_view = _strided2(wcat[:, 2 : 2 + S], 53, S)
        ys = xpack[:, S : S + NREN]
        jren = 0
        pending = False
        for k in range(1, NSTEP + 1):
            off = (k - 1) * 2 * S
            ek = et[:, off : off + 2 * S].rearrange(
                "p (two s) -> p two s", two=2
            )
            if pending:
                nc.vector.scalar_tensor_tensor(
                    w_view, _dup_free(y_sb[:], 2), inv[:], ek, MULT, MULT
                )
                pending = False
            else:
                nc.vector.tensor_mul(w_view, _dup_free(y_sb[:], 2), ek)
            nc.vector.tensor_add(u_t[:], wcat[:, 2 : 2 + S], wcat[:, 1 : 1 + S])
            nc.vector.tensor_add(y_sb[:], u_t[:], wcat[:, S + 2 : 2 * S + 2])
            if k % RENORM == 0 and k < NSTEP:
                nc.vector.reduce_max(ys[:, jren : jren + 1], y_sb[:],
                                     axis=mybir.AxisListType.X)
                nc.vector.reciprocal(inv[:], ys[:, jren : jren + 1])
                pending = True
                jren += 1

        # final multiply (slot 64 A-half: fwd E_63 / bwd ones) + renorm
        nc.vector.reduce_max(ys[:, jren : jren + 1], y_sb[:],
                             axis=mybir.AxisListType.X)
        nc.vector.reciprocal(inv[:], ys[:, jren : jren + 1])
        jren += 1
        assert jren == NREN
        efin = et[:, NSTEP * 2 * S : NSTEP * 2 * S + S]
        nc.vector.scalar_tensor_tensor(
            xpack[:, 0:S], y_sb[:], inv[:], efin, MULT, MULT
        )
        nc.sync.dma_start(xpk, xpack[:])

        # --- denominator stream (the memory-bound part) ---
        # One exp+accum per full sample; the exp image goes to a scratch
        # tile nobody reads (only the per-partition accumulator matters).
        scratch = cpool.tile([T, C], F32)
        denp = cpool.tile([T, NCOL], F32)
        for i, xt in enumerate(xts):
            cw = xt.shape[1]
            nc.scalar.activation(
                scratch[:, 0:cw], xt[:], EXP, accum_out=denp[:, i : i + 1]
            )
            if i == NCOL - 2:
                # ship everything but the final column under the last exp
                nc.sync.dma_start(dsum[:, 0 : NCOL - 1], denp[:, 0 : NCOL - 1])
        nc.sync.dma_start(dsum[:, NCOL - 1 : NCOL], denp[:, NCOL - 1 : NCOL])

    nc.compile()
    _cached["nc"] = nc
    return nc


def _host_prep(predicts, labels, label_lengths):
    predicts = np.ascontiguousarray(np.asarray(predicts, dtype=np.float32))
    labels = np.asarray(labels).astype(np.int64)
    lens = np.asarray(label_lengths).astype(np.int64)

    ext = np.zeros((B, S), np.int64)
    ext[:, 1::2] = labels
    ext_sm2 = np.zeros((B, S), np.int64)
    ext_sm2[:, 2:] = ext[:, :-2]
    skip = ((ext != 0) & (ext != ext_sm2)).astype(np.float32)  # m[s]

    g = np.take_along_axis(predicts, ext[:, None, :], axis=2)  # [B,T,S] f32
    se = (2 * lens).astype(np.int64)
    for b in range(B):
        g[b, :, se[b] + 1 :] = -1e30  # s>2*len never feeds back

    endm = np.zeros((B, S), np.float32)
    endm[np.arange(B), se] = 1.0
    endm[np.arange(B), se - 1] = 1.0

    eg = np.exp(g)  # linear-space chain factors (exp(-1e30) -> 0)
    in_maps = []
    for m in range(M):
        p = m // 2
        sl = slice(16 * p, 16 * p + PS)       # pair samples
        ep, skp, enp = eg[sl], skip[sl], endm[sl]
        ec = np.zeros((PS, NSLOT, 2, S), np.float32)
        yi = np.zeros((PS, S), np.float32)
        if m % 2 == 0:
            # forward: step k consumes E_{k-1}; A=E[k-1,s]; C=E[k-1,s'] masked
            for k in range(1, NSTEP + 1):
                ec[:, k - 1, 0, :] = ep[:, k - 1, :]
                ec[:, k - 1, 1, : S - 2] = np.where(
                    skp[:, 2:] > 0, ep[:, k - 1, : S - 2], 0.0
                )
            ec[:, NSTEP, 0, :] = ep[:, NSTEP, :]  # final-mul slot: E_63
            yi[:, 0] = 1.0
            yi[:, 1] = 1.0
        else:
            # backward, s-reversed; init absorbs E_127; steps consume E_126..E_64
            er = ep[:, :, ::-1]               # \hat E
            mr = skp[:, ::-1]                 # \hat m
            for k in range(1, NSTEP + 1):
                t = T - 2 - k                 # 125 .. 63; consumes E_{t+1}
                ec[:, k - 1, 0, :] = er[:, t + 1, :]
                ec[:, k - 1, 1, :] = np.where(mr > 0, er[:, t + 1, :], 0.0)
            ec[:, NSTEP, 0, :] = 1.0          # final-mul slot: ones
            w = ep[:, T - 1, :] * enp
            wm = skp * w
            gm = w.copy()
            gm[:, : S - 1] += w[:, 1:]
            gm[:, : S - 2] += wm[:, 2:]
            yi[:] = gm[:, ::-1]
        in_maps.append({
            "x": np.ascontiguousarray(predicts[m * BS : (m + 1) * BS]),
            "ecat": np.ascontiguousarray(ec.reshape(PS, NSLOT * 2 * S)),
            "yinit": yi,
        })
    return in_maps


def _run(in_maps, trace=False):
    nc = _build()
    res = run_bass_kernel_spmd(nc, in_maps, list(range(M)), trace=trace)
    losses = np.zeros(B, np.float32)
    for p in range(M // 2):
        re_, ro_ = res.results[2 * p], res.results[2 * p + 1]
        xe, xo = re_["xpk"][:, 0:S], ro_["xpk"][:, 0:S]
        yse, yso = re_["xpk"][:, S:], ro_["xpk"][:, S:]
        lv = (xe * xo[:, ::-1]).sum(axis=1, dtype=np.float32)
        tot = (np.log(lv) + np.log(yse).sum(1, dtype=np.float32)
               + np.log(yso).sum(1, dtype=np.float32))
        for q, r_ in ((0, re_), (1, ro_)):
            dp = r_["dsum"]  # [T, NCOL]: cols 0..BS-2 full, rest tail chunks
            den = np.concatenate(
                [dp[:, : BS - 1], dp[:, BS - 1 :].sum(1, keepdims=True)], axis=1
            )
            lnden = np.log(den).sum(axis=0, dtype=np.float32)  # [BS]
            losses[16 * p + 8 * q : 16 * p + 8 * q + BS] = (
                lnden - tot[8 * q : 8 * q + BS]
            )
    losses = np.where(losses < 1e29, losses, 0.0).astype(np.float32)
    out = np.asarray(losses.mean(), dtype=np.float32)
    return out, res


def kernel(predicts, labels, label_lengths):
    in_maps = _host_prep(predicts, labels, label_lengths)
    out, _ = _run(in_maps, trace=False)
    return out


def kernel_traced(predicts, labels, label_lengths):
    in_maps = _host_prep(predicts, labels, label_lengths)
    return _run(in_maps, trace=True)


# revision 10
# speedup vs baseline: 1.0029x; 1.0029x over previous
"""CTC loss on 8 trn2 NeuronCores.

Design:
- Batch B=64 split 8/core for the memory-bound part: each core streams its
  own 27MB of predicts through ACT exp(+accum) for the log_softmax
  denominators, which factor out of the CTC DP entirely
  (loss = -(ln L + renorms - sum_t ln denom_t)). Raw per-(t,b) denominator
  sums ship back to host; ln+sum over t happens there (keeps the ACT
  engine on the exp table set the whole kernel - no table-load thrash).
- The T=128-step CTC DP runs in linear space with sparse renorms (k=21,42
  + pre-final; f32 headroom easily covers ~5x/step growth over 21 steps).
  The serial chain is split in half across core pairs: even cores run the
  FORWARD chain for the pair's 16 samples, odd cores the BACKWARD
  (suffix) chain, both as the *identical* SPMD program - the direction
  lives entirely in host-prepared data (s-axis reversed for backward,
  transition masks baked in as zeroed linear factors, E_127 absorbed into
  the backward init). Chain factors are exp'd on host and DMA'd in ready
  to use, so the chain starts ~3us in and hides fully under the stream.
- Both chains are 63 steps of 3 fused DVE ops; cores return chain states
  via DMA and the host combines L = sum_s alpha_63[s] * gamma_63[s].
"""

from contextlib import ExitStack

import numpy as np

import concourse.bacc as bacc
import concourse.tile as tile
import concourse.mybir as mybir
from concourse.ap import AP
from concourse.bass_utils import run_bass_kernel_spmd

B, T, C, L = 64, 128, 6625, 25
S = 2 * L + 1  # 51
M = 8          # cores
BS = B // M    # own samples per core (denominator stream)
PS = 2 * BS    # pair samples per core (DP chain)
NSTEP = 63
NSLOT = 64     # 63 steps + final-multiply slot
RENORM = 21    # in-chain renorm every 21 steps (k=21, 42)
NREN = 3       # 2 in-chain renorms + 1 pre-final
# Last sample streams in shrinking column chunks so the final exp (which
# can only start once the last bytes land) is short.
TAILW = [2625, 1700, 1500, 800]
NCOL = BS - 1 + len(TAILW)  # denominator accumulator columns
F32 = mybir.dt.float32

_cached = {}


def _dup_free(ap, n):
    """AP reading the free range of `ap` n times: [.., (0,n), (step,cnt)]."""
    dims = [list(d) for d in ap.ap]
    return AP(ap.tensor, ap.offset, dims[:-1] + [[0, n]] + [dims[-1]])


def _strided2(ap, gap, n):
    """AP over `ap`'s tensor writing two n-wide blocks `gap` apart."""
    dims = [list(d) for d in ap.ap]
    return AP(ap.tensor, ap.offset, dims[:-1] + [[gap, 2], [1, n]])


def _build():
    if "nc" in _cached:
        return _cached["nc"]
    nc = bacc.Bacc(
        "TRN2", target_bir_lowering=False, debug=False, num_devices=M
    )
    x = nc.dram_tensor("x", [BS, T, C], F32, kind="ExternalInput").ap()
    ecat = nc.dram_tensor("ecat", [PS, NSLOT * 2 * S], F32,
                          kind="ExternalInput").ap()
    yinit = nc.dram_tensor("yinit", [PS, S], F32, kind="ExternalInput").ap()
    xpk = nc.dram_tensor("xpk", [PS, S + NREN], F32, kind="ExternalOutput").ap()
    dsum = nc.dram_tensor("dsum", [T, NCOL], F32, kind="ExternalOutput").ap()

    EXP = mybir.ActivationFunctionType.Exp
    MULT = mybir.AluOpType.mult

    with tile.TileContext(nc) as tc, ExitStack() as ctx:
        cpool = ctx.enter_context(tc.tile_pool(name="consts", bufs=1))
        xpool = ctx.enter_context(tc.tile_pool(name="xs", bufs=4))

        # --- chain factors + init first (chain hides under the stream) ---
        et = cpool.tile([PS, NSLOT * 2 * S], F32)
        nc.sync.dma_start(et[:], ecat)
        y_sb = cpool.tile([PS, S], F32)
        nc.sync.dma_start(y_sb[:], yinit)

        # --- arm the stream: 7 full samples + shrinking tail chunks ---
        # The first sample goes in four partition-quarters ordered so that
        # descriptor generation (serial, partition order) reaches all 16
        # SDMA engines early: 0-63 feed even engines, 64-127 odd ones.
        xts = []
        for b in range(BS - 1):
            xt = xpool.tile([T, C], F32, tag="xt")
            if b == 0:
                for p0, p1 in ((0, 32), (64, 96), (32, 64), (96, 128)):
                    nc.sync.dma_start(xt[p0:p1, :], x[b, p0:p1, :])
            else:
                nc.sync.dma_start(xt[:], x[b])
            xts.append(xt)
        c0 = 0
        for cw in TAILW:
            xt = xpool.tile([T, cw], F32, tag="xt")
            nc.sync.dma_start(xt[:], x[BS - 1, :, c0 : c0 + cw])
            xts.append(xt)
            c0 += cw

        # --- DP chain: 63 steps of 3 fused DVE ops ---
        # wcat layout: [pad2 | w(51) | pad2 | wc(51)] = 106 cols
        wcat = cpool.tile([PS, 2 * S + 4], F32)
        u_t = cpool.tile([PS, S], F32)
        xpack = cpool.tile([PS, S + NREN], F32)  # [X(51) | ys(NREN)]
        inv = cpool.tile([PS, 1], F32)
        nc.vector.memset(wcat[:], 0.0)

        w_view = _strided2(wcat[:, 2 : 2 + S], 53, S)
        ys = xpack[:, S : S + NREN]
        jren = 0
        pending = False
        for k in range(1, NSTEP + 1):
            off = (k - 1) * 2 * S
            ek = et[:, off : off + 2 * S].rearrange(
                "p (two s) -> p two s", two=2
            )
            if pending:
                nc.vector.scalar_tensor_tensor(
                    w_view, _dup_free(y_sb[:], 2), inv[:], ek, MULT, MULT
                )
                pending = False
            else:
                nc.vector.tensor_mul(w_view, _dup_free(y_sb[:], 2), ek)
            nc.vector.tensor_add(u_t[:], wcat[:, 2 : 2 + S], wcat[:, 1 : 1 + S])
            nc.vector.tensor_add(y_sb[:], u_t[:], wcat[:, S + 2 : 2 * S + 2])
            if k % RENORM == 0 and k < NSTEP:
                nc.vector.reduce_max(ys[:, jren : jren + 1], y_sb[:],
                                     axis=mybir.AxisListType.X)
                nc.vector.reciprocal(inv[:], ys[:, jren : jren + 1])
                pending = True
                jren += 1

        # final multiply (slot 64 A-half: fwd E_63 / bwd ones) + renorm
        nc.vector.reduce_max(ys[:, jren : jren + 1], y_sb[:],
                             axis=mybir.AxisListType.X)
        nc.vector.reciprocal(inv[:], ys[:, jren : jren + 1])
        jren += 1
        assert jren == NREN
        efin = et[:, NSTEP * 2 * S : NSTEP * 2 * S + S]
        nc.vector.scalar_tensor_tensor(
            xpack[:, 0:S], y_sb[:], inv[:], efin, MULT, MULT
        )
        nc.sync.dma_start(xpk, xpack[:])

        # --- denominator stream (the memory-bound part) ---
        # One exp+accum per full sample; the exp image goes to a scratch
        # tile nobody reads (only the per-partition accumulator matters).
        scratch = cpool.tile([T, C], F32)
        denp = cpool.tile([T, NCOL], F32)
        for i, xt in enumerate(xts):
            cw = xt.shape[1]
            nc.scalar.activation(
                scratch[:, 0:cw], xt[:], EXP, accum_out=denp[:, i : i + 1]
            )
            if i == NCOL - 2:
                # ship everything but the final column under the last exp
                nc.sync.dma_start(dsum[:, 0 : NCOL - 1], denp[:, 0 : NCOL - 1])
        nc.sync.dma_start(dsum[:, NCOL - 1 : NCOL], denp[:, NCOL - 1 : NCOL])

    nc.compile()
    _cached["nc"] = nc
    return nc


def _host_prep(predicts, labels, label_lengths):
    predicts = np.ascontiguousarray(np.asarray(predicts, dtype=np.float32))
    labels = np.asarray(labels).astype(np.int64)
    lens = np.asarray(label_lengths).astype(np.int64)

    ext = np.zeros((B, S), np.int64)
    ext[:, 1::2] = labels
    ext_sm2 = np.zeros((B, S), np.int64)
    ext_sm2[:, 2:] = ext[:, :-2]
    skip = ((ext != 0) & (ext != ext_sm2)).astype(np.float32)  # m[s]

    g = np.take_along_axis(predicts, ext[:, None, :], axis=2)  # [B,T,S] f32
    se = (2 * lens).astype(np.int64)
    for b in range(B):
        g[b, :, se[b] + 1 :] = -1e30  # s>2*len never feeds back

    endm = np.zeros((B, S), np.float32)
    endm[np.arange(B), se] = 1.0
    endm[np.arange(B), se - 1] = 1.0

    eg = np.exp(g)  # linear-space chain factors (exp(-1e30) -> 0)
    in_maps = []
    for m in range(M):
        p = m // 2
        sl = slice(16 * p, 16 * p + PS)       # pair samples
        ep, skp, enp = eg[sl], skip[sl], endm[sl]
        ec = np.zeros((PS, NSLOT, 2, S), np.float32)
        yi = np.zeros((PS, S), np.float32)
        if m % 2 == 0:
            # forward: step k consumes E_{k-1}; A=E[k-1,s]; C=E[k-1,s'] masked
            for k in range(1, NSTEP + 1):
                ec[:, k - 1, 0, :] = ep[:, k - 1, :]
                ec[:, k - 1, 1, : S - 2] = np.where(
                    skp[:, 2:] > 0, ep[:, k - 1, : S - 2], 0.0
                )
            ec[:, NSTEP, 0, :] = ep[:, NSTEP, :]  # final-mul slot: E_63
            yi[:, 0] = 1.0
            yi[:, 1] = 1.0
        else:
            # backward, s-reversed; init absorbs E_127; steps consume E_126..E_64
            er = ep[:, :, ::-1]               # \hat E
            mr = skp[:, ::-1]                 # \hat m
            for k in range(1, NSTEP + 1):
                t = T - 2 - k                 # 125 .. 63; consumes E_{t+1}
                ec[:, k - 1, 0, :] = er[:, t + 1, :]
                ec[:, k - 1, 1, :] = np.where(mr > 0, er[:, t + 1, :], 0.0)
            ec[:, NSTEP, 0, :] = 1.0          # final-mul slot: ones
            w = ep[:, T - 1, :] * enp
            wm = skp * w
            gm = w.copy()
            gm[:, : S - 1] += w[:, 1:]
            gm[:, : S - 2] += wm[:, 2:]
            yi[:] = gm[:, ::-1]
        in_maps.append({
            "x": np.ascontiguousarray(predicts[m * BS : (m + 1) * BS]),
            "ecat": np.ascontiguousarray(ec.reshape(PS, NSLOT * 2 * S)),
            "yinit": yi,
        })
    return in_maps


def _run(in_maps, trace=False):
    nc = _build()
    res = run_bass_kernel_spmd(nc, in_maps, list(range(M)), trace=trace)
    losses = np.zeros(B, np.float32)
    for p in range(M // 2):
        re_, ro_ = res.results[2 * p], res.results[2 * p + 1]
        xe, xo = re_["xpk"][:, 0:S], ro_["xpk"][:, 0:S]
        yse, yso = re_["xpk"][:, S:], ro_["xpk"][:, S:]
        lv = (xe * xo[:, ::-1]).sum(axis=1, dtype=np.float32)
        tot = (np.log(lv) + np.log(yse).sum(1, dtype=np.float32)
               + np.log(yso).sum(1, dtype=np.float32))
        for q, r_ in ((0, re_), (1, ro_)):
            dp = r_["dsum"]  # [T, NCOL]: cols 0..BS-2 full, rest tail chunks
            den = np.concatenate(
                [dp[:, : BS - 1], dp[:, BS - 1 :].sum(1, keepdims=True)], axis=1
            )
            lnden = np.log(den).sum(axis=0, dtype=np.float32)  # [BS]
            losses[16 * p + 8 * q : 16 * p + 8 * q + BS] = (
                lnden - tot[8 * q : 8 * q + BS]
            )
    losses = np.where(losses < 1e29, losses, 0.0).astype(np.float32)
    out = np.asarray(losses.mean(), dtype=np.float32)
    return out, res


def kernel(predicts, labels, label_lengths):
    in_maps = _host_prep(predicts, labels, label_lengths)
    out, _ = _run(in_maps, trace=False)
    return out


def kernel_traced(predicts, labels, label_lengths):
    in_maps = _host_prep(predicts, labels, label_lengths)
    return _run(in_maps, trace=True)


# revision 11
# speedup vs baseline: 1.0966x; 1.0935x over previous
"""CTC loss on 8 trn2 NeuronCores.

Design:
- Batch B=64 split 8/core for the memory-bound part: each core streams its
  own 27MB of predicts through ACT exp(+accum) for the log_softmax
  denominators, which factor out of the CTC DP entirely
  (loss = -(ln L + renorms - sum_t ln denom_t)). Raw per-(t,b) denominator
  sums ship back to host; ln+sum over t happens there (keeps the ACT
  engine on the exp table set the whole kernel - no table-load thrash).
- The T=128-step CTC DP runs in linear space with sparse renorms (k=21,42
  + pre-final; f32 headroom easily covers ~5x/step growth over 21 steps).
  The serial chain is split in half across core pairs: even cores run the
  FORWARD chain for the pair's 16 samples, odd cores the BACKWARD
  (suffix) chain, both as the *identical* SPMD program - the direction
  lives entirely in host-prepared data (s-axis reversed for backward,
  transition masks baked in as zeroed linear factors, E_127 absorbed into
  the backward init). Chain factors are exp'd on host and DMA'd in ready
  to use, so the chain starts ~3us in and hides fully under the stream.
- Both chains are 63 steps of 3 fused DVE ops; cores return chain states
  via DMA and the host combines L = sum_s alpha_63[s] * gamma_63[s].
"""

from contextlib import ExitStack

import numpy as np

import concourse.bacc as bacc
import concourse.tile as tile
import concourse.mybir as mybir
from concourse.ap import AP
from concourse.bass_utils import run_bass_kernel_spmd

B, T, C, L = 64, 128, 6625, 25
S = 2 * L + 1  # 51
M = 8          # cores
BS = B // M    # own samples per core (denominator stream)
PS = 2 * BS    # pair samples per core (DP chain)
NSTEP = 63
NSLOT = 64     # 63 steps + final-multiply slot
RENORM = 21    # in-chain renorm every 21 steps (k=21, 42)
NREN = 3       # 2 in-chain renorms + 1 pre-final
# Last sample streams in shrinking column chunks so the final exp (which
# can only start once the last bytes land) is short.
TAILW = [2625, 1700, 1500, 800]
NCOL = BS - 1 + len(TAILW)  # denominator accumulator columns
F32 = mybir.dt.float32

_cached = {}


def _dup_free(ap, n):
    """AP reading the free range of `ap` n times: [.., (0,n), (step,cnt)]."""
    dims = [list(d) for d in ap.ap]
    return AP(ap.tensor, ap.offset, dims[:-1] + [[0, n]] + [dims[-1]])


def _strided2(ap, gap, n):
    """AP over `ap`'s tensor writing two n-wide blocks `gap` apart."""
    dims = [list(d) for d in ap.ap]
    return AP(ap.tensor, ap.offset, dims[:-1] + [[gap, 2], [1, n]])


def _build():
    if "nc" in _cached:
        return _cached["nc"]
    nc = bacc.Bacc(
        "TRN2", target_bir_lowering=False, debug=False, num_devices=M
    )
    x = nc.dram_tensor("x", [BS, T, C], F32, kind="ExternalInput").ap()
    ecat = nc.dram_tensor("ecat", [PS, NSLOT * 2 * S], F32,
                          kind="ExternalInput").ap()
    yinit = nc.dram_tensor("yinit", [PS, S], F32, kind="ExternalInput").ap()
    xpk = nc.dram_tensor("xpk", [PS, S + NREN], F32, kind="ExternalOutput").ap()
    dsum = nc.dram_tensor("dsum", [T, NCOL], F32, kind="ExternalOutput").ap()

    EXP = mybir.ActivationFunctionType.Exp
    MULT = mybir.AluOpType.mult

    with tile.TileContext(nc) as tc, ExitStack() as ctx:
        cpool = ctx.enter_context(tc.tile_pool(name="consts", bufs=1))
        xpool = ctx.enter_context(tc.tile_pool(name="xs", bufs=4))

        # --- chain factors + init first (chain hides under the stream) ---
        et = cpool.tile([PS, NSLOT * 2 * S], F32)
        nc.sync.dma_start(et[:], ecat)
        y_sb = cpool.tile([PS, S], F32)
        nc.sync.dma_start(y_sb[:], yinit)

        # --- arm the stream: 7 full samples + shrinking tail chunks ---
        xts = []
        for b in range(BS - 1):
            xt = xpool.tile([T, C], F32, tag="xt")
            nc.sync.dma_start(xt[:], x[b])
            xts.append(xt)
        c0 = 0
        for cw in TAILW:
            xt = xpool.tile([T, cw], F32, tag="xt")
            nc.sync.dma_start(xt[:], x[BS - 1, :, c0 : c0 + cw])
            xts.append(xt)
            c0 += cw

        # --- DP chain: 63 steps of 3 fused DVE ops ---
        # wcat layout: [pad2 | w(51) | pad2 | wc(51)] = 106 cols
        wcat = cpool.tile([PS, 2 * S + 4], F32)
        u_t = cpool.tile([PS, S], F32)
        xpack = cpool.tile([PS, S + NREN], F32)  # [X(51) | ys(NREN)]
        inv = cpool.tile([PS, 1], F32)
        nc.vector.memset(wcat[:], 0.0)

        w_view = _strided2(wcat[:, 2 : 2 + S], 53, S)
        ys = xpack[:, S : S + NREN]
        jren = 0
        pending = False
        for k in range(1, NSTEP + 1):
            off = (k - 1) * 2 * S
            ek = et[:, off : off + 2 * S].rearrange(
                "p (two s) -> p two s", two=2
            )
            if pending:
                nc.vector.scalar_tensor_tensor(
                    w_view, _dup_free(y_sb[:], 2), inv[:], ek, MULT, MULT
                )
                pending = False
            else:
                nc.vector.tensor_mul(w_view, _dup_free(y_sb[:], 2), ek)
            nc.vector.tensor_add(u_t[:], wcat[:, 2 : 2 + S], wcat[:, 1 : 1 + S])
            nc.vector.tensor_add(y_sb[:], u_t[:], wcat[:, S + 2 : 2 * S + 2])
            if k % RENORM == 0 and k < NSTEP:
                nc.vector.reduce_max(ys[:, jren : jren + 1], y_sb[:],
                                     axis=mybir.AxisListType.X)
                nc.vector.reciprocal(inv[:], ys[:, jren : jren + 1])
                pending = True
                jren += 1

        # final multiply (slot 64 A-half: fwd E_63 / bwd ones) + renorm
        nc.vector.reduce_max(ys[:, jren : jren + 1], y_sb[:],
                             axis=mybir.AxisListType.X)
        nc.vector.reciprocal(inv[:], ys[:, jren : jren + 1])
        jren += 1
        assert jren == NREN
        efin = et[:, NSTEP * 2 * S : NSTEP * 2 * S + S]
        nc.vector.scalar_tensor_tensor(
            xpack[:, 0:S], y_sb[:], inv[:], efin, MULT, MULT
        )
        nc.sync.dma_start(xpk, xpack[:])

        # --- denominator stream (the memory-bound part) ---
        # One exp+accum per full sample; the exp image goes to a scratch
        # tile nobody reads (only the per-partition accumulator matters).
        scratch = cpool.tile([T, C], F32)
        denp = cpool.tile([T, NCOL], F32)
        for i, xt in enumerate(xts):
            cw = xt.shape[1]
            nc.scalar.activation(
                scratch[:, 0:cw], xt[:], EXP, accum_out=denp[:, i : i + 1]
            )
            if i == NCOL - 2:
                # ship everything but the final column under the last exp
                nc.sync.dma_start(dsum[:, 0 : NCOL - 1], denp[:, 0 : NCOL - 1])
        nc.sync.dma_start(dsum[:, NCOL - 1 : NCOL], denp[:, NCOL - 1 : NCOL])

    nc.compile()
    _cached["nc"] = nc
    return nc


def _host_prep(predicts, labels, label_lengths):
    predicts = np.ascontiguousarray(np.asarray(predicts, dtype=np.float32))
    labels = np.asarray(labels).astype(np.int64)
    lens = np.asarray(label_lengths).astype(np.int64)

    ext = np.zeros((B, S), np.int64)
    ext[:, 1::2] = labels
    ext_sm2 = np.zeros((B, S), np.int64)
    ext_sm2[:, 2:] = ext[:, :-2]
    skip = ((ext != 0) & (ext != ext_sm2)).astype(np.float32)  # m[s]

    g = np.take_along_axis(predicts, ext[:, None, :], axis=2)  # [B,T,S] f32
    se = (2 * lens).astype(np.int64)
    for b in range(B):
        g[b, :, se[b] + 1 :] = -1e30  # s>2*len never feeds back

    endm = np.zeros((B, S), np.float32)
    endm[np.arange(B), se] = 1.0
    endm[np.arange(B), se - 1] = 1.0

    eg = np.exp(g)  # linear-space chain factors (exp(-1e30) -> 0)
    in_maps = []
    for m in range(M):
        p = m // 2
        sl = slice(16 * p, 16 * p + PS)       # pair samples
        ep, skp, enp = eg[sl], skip[sl], endm[sl]
        ec = np.zeros((PS, NSLOT, 2, S), np.float32)
        yi = np.zeros((PS, S), np.float32)
        if m % 2 == 0:
            # forward: step k consumes E_{k-1}; A=E[k-1,s]; C=E[k-1,s'] masked
            for k in range(1, NSTEP + 1):
                ec[:, k - 1, 0, :] = ep[:, k - 1, :]
                ec[:, k - 1, 1, : S - 2] = np.where(
                    skp[:, 2:] > 0, ep[:, k - 1, : S - 2], 0.0
                )
            ec[:, NSTEP, 0, :] = ep[:, NSTEP, :]  # final-mul slot: E_63
            yi[:, 0] = 1.0
            yi[:, 1] = 1.0
        else:
            # backward, s-reversed; init absorbs E_127; steps consume E_126..E_64
            er = ep[:, :, ::-1]               # \hat E
            mr = skp[:, ::-1]                 # \hat m
            for k in range(1, NSTEP + 1):
                t = T - 2 - k                 # 125 .. 63; consumes E_{t+1}
                ec[:, k - 1, 0, :] = er[:, t + 1, :]
                ec[:, k - 1, 1, :] = np.where(mr > 0, er[:, t + 1, :], 0.0)
            ec[:, NSTEP, 0, :] = 1.0          # final-mul slot: ones
            w = ep[:, T - 1, :] * enp
            wm = skp * w
            gm = w.copy()
            gm[:, : S - 1] += w[:, 1:]
            gm[:, : S - 2] += wm[:, 2:]
            yi[:] = gm[:, ::-1]
        in_maps.append({
            "x": np.ascontiguousarray(predicts[m * BS : (m + 1) * BS]),
            "ecat": np.ascontiguousarray(ec.reshape(PS, NSLOT * 2 * S)),
            "yinit": yi,
        })
    return in_maps


def _run(in_maps, trace=False):
    nc = _build()
    res = run_bass_kernel_spmd(nc, in_maps, list(range(M)), trace=trace)
    losses = np.zeros(B, np.float32)
    for p in range(M // 2):
        re_, ro_ = res.results[2 * p], res.results[2 * p + 1]
        xe, xo = re_["xpk"][:, 0:S], ro_["xpk"][:, 0:S]
        yse, yso = re_["xpk"][:, S:], ro_["xpk"][:, S:]
        lv = (xe * xo[:, ::-1]).sum(axis=1, dtype=np.float32)
        tot = (np.log(lv) + np.log(yse).sum(1, dtype=np.float32)
               + np.log(yso).sum(1, dtype=np.float32))
        for q, r_ in ((0, re_), (1, ro_)):
            dp = r_["dsum"]  # [T, NCOL]: cols 0..BS-2 full, rest tail chunks
            den = np.concatenate(
                [dp[:, : BS - 1], dp[:, BS - 1 :].sum(1, keepdims=True)], axis=1
            )
            lnden = np.log(den).sum(axis=0, dtype=np.float32)  # [BS]
            losses[16 * p + 8 * q : 16 * p + 8 * q + BS] = (
                lnden - tot[8 * q : 8 * q + BS]
            )
    losses = np.where(losses < 1e29, losses, 0.0).astype(np.float32)
    out = np.asarray(losses.mean(), dtype=np.float32)
    return out, res


def kernel(predicts, labels, label_lengths):
    in_maps = _host_prep(predicts, labels, label_lengths)
    out, _ = _run(in_maps, trace=False)
    return out


def kernel_traced(predicts, labels, label_lengths):
    in_maps = _host_prep(predicts, labels, label_lengths)
    return _run(in_maps, trace=True)
